# revision 51
# baseline (speedup 1.0000x reference)
"""MoE QLoRA linear kernel for Trainium2 (8 NeuronCores, data-parallel over tokens).

Computes, for x:(B,S,IN) f32:
    base  = x @ W.T + b
    gates = softmax(x @ Wr.T)                       # (tok, E)
    proj  = x @ A[e].T                              # (tok, E, R)
    out   = base + sum_e SCALE * gates[...,e] * (proj[...,e,:] @ Bm[e].T)

Key algebraic fold: the gated expert mix is a single rank-(E*R) matmul:
    wproj[t, er] = SCALE * gates[t, e] * proj[t, er]          (er = e*R+r)
    lora[t, o]   = sum_er wproj[t, er] * Bcat[er, o]          (Bcat[er,o] = Bm[e,o,r])
and the bias b is folded in as an extra contraction row (wproj row of ones,
Bcat row = b), so base+lora+bias all accumulate in one PSUM group on the PE.

Per-core kernel (1024 tokens), everything oriented (feature-partition, token-free):
  phase 1: PSUM(36,512) = [A;Wr]^T-stationary matmuls over 32 k-tiles ->
           proj rows 0..31, router logits rows 32..35 (col-tiled: the two
           token slabs stream concurrently); softmax via exp + staged PE
           ones-matmul partition reductions/broadcasts; wproj written fp16.
           The o-tile-0 base k-loop is interleaved into the proj k-loop
           (k-index staggered behind it) so the PE tracks the x DMA stream,
           and o-tile 1's k-loop is sliced between the gating stages.
  phase 2: for each of 32 o-tiles: out(128o, t) = W-tile-stationary matmul
           over 32 k-tiles + one lora matmul (k=33) accumulated into PSUM,
           copy to SBUF, DMA out as (OUT, tok); host transposes back.

All matmul inputs are fp16 (host-cast; PE runs fp16 at full bf16 rate,
fp32 PSUM accumulation). Host pre-tiles all layouts so every DMA is
contiguous and the kernel needs zero on-chip transposes.

Optimizations landed on top of the first working version (600us/core):
 - LDWEIGHTS dedupe (_dedupe_ldweights): walrus runs --enable-ldw-opt=false
   (=true crashes its codegen on our standalone LDWs), so each matmul gets
   its own LDWEIGHTS; the second load of each same-stationary (o-tile, k)
   pair is redundant. Deleting them post-compile is hardware-validated and
   worth ~30us/rep (probe-measured). The signature includes tile_position/
   tile_size, and any self-loading (fp32) matmul resets it — without that,
   two deletions with an interleaved gating matmul corrupted o-tile 1.
 - fp16 gating matmuls (fp32 ran at 1/4 PE rate): ~4us.
 - Row-tiled lora tails: slab 0 in PE rows 0..32, slab 1 in rows 64..96
   (bt/wp duplicated at partition offset 64), so the two 512-cycle lora
   matmuls stream concurrently: ~7us.
 - Col-tiled phase-1 proj: slab 0 -> PSUM partitions 0..35 (PE col group 0),
   slab 1 -> partitions 64..99 of its own bank (col group 64), sharing the
   art stationary: ~7us.
 - DMA issue order tracks PE consumption (art k0-3, x k0, W0, ... W two
   o-tiles ahead of the PE), and the startup k-loop interleaves proj with
   o-tile 0 so the PE follows the x DMA stream: ~6us of startup idle.
 - Staged gating: the softmax chain is a serial PE->DVE->PE ping-pong, so
   its stages are emitted with slices of o-tile 1's k-loop between them
   (every cross-engine hop hides under ~1.7us of base matmuls), and all six
   mini-matmul outputs share one manually-sliced PSUM bank at 32-aligned
   partition offsets (single-shot matmuls: a later start's whole-bank
   has_written clear resets accumulate semantics, never stored data): ~4us.

Measured device behavior (pure-matmul probe, ldw_probe.py): the chip
oscillates between ~2.4GHz and ~2.0GHz PE power states with ~0.1-1s dwell;
per-rep slope is ~446us fast / ~528us slow with dedupe (the fp16 streaming
roofline for 2112 N=512 matmuls is 450us at 2.4GHz, i.e. LDWEIGHTS is fully
hidden after dedupe). test.py therefore reports the median per-rep slope
over many short alternating loops, scaled by the cost-model full/base ratio.
"""

import numpy as np

import concourse.bass as bass
import concourse.tile as tile
from concourse import bacc, mybir
from concourse import bass_utils

# Problem shape (hardcoded; kernel.py must be self-contained)
B, S, IN, OUT, E, R = 4, 2048, 4096, 4096, 4, 8
SCALE = 16.0 / 8.0
N_CORES = 8
TOK = B * S                  # 8192 tokens
TPC = TOK // N_CORES         # 1024 tokens per core
P = 128                      # partitions
KT = IN // P                 # 32 k-tiles (contraction)
OT = OUT // P                # 32 output tiles
NSLAB = 512                  # moving-operand free size (PSUM bank = 512 f32)
NS = TPC // NSLAB            # 2 token slabs per core
ER = E * R                   # 32 low-rank rows
ERA = ER + 1                 # +1 ones row (bias fold)
ROWB = 64                    # partition offset of slab-1's lora row-tile

F16 = mybir.dt.float16
F32 = mybir.dt.float32

_NC = None

# Post-compile pass: delete redundant consecutive InstLdweights from the PE
# stream. Walrus (--enable-ldw-opt=false) emits one LDWEIGHTS per matmul;
# when consecutive matmuls share the same stationary tile (the two token
# slabs of each (o-tile, k)), the second load is identical, wait/update-free,
# and costs ~53ns of serialized PE time. Deleting it lets the following
# non-self-loading matmul reuse the already-loaded weights.
DEDUPE_LDW = True


def _dedupe_ldweights(nc):
    removed = 0
    for blk in nc.m.functions[0].blocks:
        keep = []
        prev_sig = None
        for inst in blk.instructions:
            if isinstance(inst, mybir.InstLdweights):
                sig = (
                    str(inst.ins[0]),
                    str(inst.tile_position),
                    str(inst.tile_size),
                    str(inst.perf_mode),
                    str(inst.is_transpose),
                )
                if sig == prev_sig and not inst.has_wait() and not inst.has_update():
                    removed += 1
                    continue
                prev_sig = sig
            elif isinstance(inst, mybir.InstMatmult) and inst.ldweights is not False:
                # self-loading matmul (fp32 gating) reloads the PE array's
                # stationary weights — following matmuls need a fresh LDW
                prev_sig = None
            keep.append(inst)
        blk.instructions = keep
    return removed


def build_nc(reps=1, ns=NS):
    NS_ = ns
    nc = bacc.Bacc("TRN2", target_bir_lowering=False, debug=False)

    xd = nc.dram_tensor("xd", [P, KT, TPC], F16, kind="ExternalInput")
    wd = nc.dram_tensor("wd", [OT, P, KT, P], F16, kind="ExternalInput")
    artd = nc.dram_tensor("artd", [P, KT, ER + E], F16, kind="ExternalInput")
    # btd rows 0..32 = [Bcat; bias]; rows 64..96 = same again, so the two
    # slabs' lora matmuls can run concurrently as PE row-tiles (0-63 / 64-127)
    btd = nc.dram_tensor("btd", [ROWB + ERA, OUT], F16, kind="ExternalInput")
    seld = nc.dram_tensor("seld", [E, ER], F16, kind="ExternalInput")
    od = nc.dram_tensor("od", [OUT, TPC], F32, kind="ExternalOutput")

    with tile.TileContext(nc) as tc:
        with (
            tc.tile_pool(name="consts", bufs=1) as consts,
            tc.tile_pool(name="wpool", bufs=4) as wpool,
            tc.tile_pool(name="opool", bufs=3) as opool,
            tc.tile_pool(name="small", bufs=2) as small,
            tc.tile_pool(name="psum_proj", bufs=1, space="PSUM") as psum_proj,
            tc.tile_pool(name="psum_base", bufs=2, space="PSUM") as psum_base,
        ):
            # DMA issue order tracks the PE consumption schedule: phase 1
            # consumes art k-tiles + x k-tiles in order from t=0; W tile 0
            # isn't needed until the o-tile-0 k-loop (~15us in), W tile 1 and
            # the bias/sel tiles later still. Keeping the big W loads out of
            # the head of the queue cuts the PE's startup DMA wait.
            art_sb = consts.tile([P, KT, ER + E], F16)
            nc.sync.dma_start(out=art_sb[:, 0:4, :], in_=artd[:, 0:4, :])

            w_tiles = {}

            def load_w(ot):
                w_sb = wpool.tile([P, KT, P], F16, tag="w", name="w_sb")
                nc.sync.dma_start(out=w_sb[:], in_=wd[ot])
                w_tiles[ot] = w_sb

            # Resident activations: x^T tiled (p=i%128, k=i//128, t), fp16, 8 MiB.
            # W0 right after x k0: the k-interleaved startup loop consumes
            # W0's k-tiles from its 4th matmul on.
            x_sb = consts.tile([P, KT, TPC], F16)
            nc.sync.dma_start(out=x_sb[:, 0, :], in_=xd[:, 0, :])
            load_w(0)
            nc.sync.dma_start(out=art_sb[:, 4:, :], in_=artd[:, 4:, :])
            for k in range(1, 20):
                nc.sync.dma_start(out=x_sb[:, k, :], in_=xd[:, k, :])
            # W1/sel/bt aren't consumed until ~30us/~45us in; keeping them
            # behind the first 20 x k-tiles keeps the startup loop x-fed
            load_w(1)
            sel_sb = consts.tile([E, ER], F16)
            nc.sync.dma_start(out=sel_sb[:], in_=seld[:])
            for k in range(20, 25):
                nc.sync.dma_start(out=x_sb[:, k, :], in_=xd[:, k, :])
            bt_sb = consts.tile([ROWB + ERA, OUT], F16)
            nc.sync.dma_start(out=bt_sb[:], in_=btd[:])
            for k in range(25, KT):
                nc.sync.dma_start(out=x_sb[:, k, :], in_=xd[:, k, :])

            ones_e1 = consts.tile([E, 1], F16)
            nc.vector.memset(ones_e1[:], 1.0)
            ones_1e = consts.tile([1, E], F16)
            nc.vector.memset(ones_1e[:], 1.0)
            # Gated low-rank projection, fp16. Slab 0 lives in rows 0..31
            # (+ones row 32), slab 1 in rows 64..95 (+ones row 96) so the two
            # lora matmuls occupy disjoint PE row groups and run concurrently.
            wp_sb = consts.tile([ROWB + ERA, TPC], F16)
            nc.vector.memset(wp_sb[ER : ER + 1, :], 1.0)
            nc.vector.memset(wp_sb[ROWB + ER : ROWB + ERA, :], 1.0)

            # ---------- phase 1: proj + router matmul PSUM tiles ----------
            # Slab 0's proj output sits at PSUM partitions 0..35 (PE col group
            # 0), slab 1's at partitions 64..99 of its own bank (col group
            # 64): the two matmuls share the art stationary but occupy
            # disjoint PE column groups, so they stream concurrently.
            # Per slab: rows +0..31 = proj^T (er), rows +32..35 router logits.
            pp_tiles = [
                psum_proj.tile(
                    [t * ROWB + ER + E, NSLAB], F32, tag=f"pp{t}", name=f"pp{t}"
                )
                for t in range(NS_)
            ]
            pps = [pp_tiles[t][t * ROWB : t * ROWB + ER + E, :] for t in range(NS_)]

            # ---------- gating: softmax over the 4 expert rows ----------
            # (no max-sub: |logit| < ~8). Partition reductions/broadcasts are
            # tiny PE matmuls (fp16 operands: fp32 ran at 1/4 rate). The chain
            # is a serial PE->DVE->PE ping-pong (sum -> recip -> bcast -> mul
            # -> select), so it is emitted in STAGES with slices of o-tile 1's
            # k-loop between them: every cross-engine hop hides under ~1.7us
            # of base matmuls instead of stalling the in-order PE.
            #
            # All six mini-matmul outputs share ONE manually-sliced PSUM bank
            # at 32-aligned partition offsets (slab 0 at 0..35, slab 1 at
            # 64..99 -> disjoint PE col groups, concurrent). This is safe
            # because each mini-matmul is single-shot (start+stop): a later
            # start's whole-bank has_written clear resets accumulate semantics
            # only, never stored data, and the one region overlap (g32 over
            # s) is ordered by the true dependency chain through recip.
            ga = [dict() for _ in range(NS_)]

            def gat_exp(t):
                g = ga[t]
                g["e"] = small.tile([E, NSLAB], F16, tag="e", name="e_sb")
                nc.scalar.activation(
                    g["e"][:],
                    pps[t][ER : ER + E, :],
                    mybir.ActivationFunctionType.Exp,
                )

            def gat_sum(t, gat_ps):
                g = ga[t]
                g["s"] = gat_ps[t * ROWB : t * ROWB + 1, :]
                nc.tensor.matmul(g["s"], ones_e1[:], g["e"][:])  # sum_e exp

            def gat_recip(t):
                g = ga[t]
                g["r"] = small.tile([1, NSLAB], F16, tag="r", name="r_sb")
                with nc.allow_low_precision(
                    reason="softmax gates are O(1) and scale only the small "
                    "lora term; fp16 is ample"
                ):
                    nc.vector.reciprocal(g["r"][:], g["s"])

            def gat_r4(t, gat_ps):
                g = ga[t]
                g["r4"] = gat_ps[t * ROWB + 32 : t * ROWB + 32 + E, :]
                # bcast to 4 rows; explicit tile_position (auto-derivation
                # rejects output base partition 96)
                nc.tensor.matmul(
                    g["r4"],
                    ones_1e[:],
                    g["r"][:],
                    tile_position=(0, t * ROWB + 32),
                )

            def gat_g4(t):
                g = ga[t]
                g["g4"] = small.tile([E, NSLAB], F16, tag="g4", name="g4_sb")
                nc.vector.tensor_mul(g["g4"][:], g["e"][:], g["r4"])

            def gat_g32(t, gat_ps):
                # (SCALE * gate)[er, t] via 0/1*SCALE selection matmul
                g = ga[t]
                g["g32"] = gat_ps[t * ROWB : t * ROWB + ER, :]
                nc.tensor.matmul(g["g32"], sel_sb[:], g["g4"][:])

            def gat_wp(t):
                g = ga[t]
                tsl = slice(t * NSLAB, (t + 1) * NSLAB)
                rb = t * ROWB  # slab-1 wproj lives at partition offset 64
                # walrus: tensor_tensor may read at most one operand from PSUM
                g32_sb = small.tile([ER, NSLAB], F32, tag="g32s", name="g32_sb")
                nc.vector.tensor_copy(g32_sb[:], g["g32"])
                nc.vector.tensor_mul(
                    wp_sb[rb : rb + ER, tsl], pps[t][0:ER, :], g32_sb[:]
                )

            # ---------- phase 2: base matmul + lora + bias ----------
            def alloc_pots():
                return [
                    psum_base.tile([P, NSLAB], F32, tag=f"po{t}", name=f"po{t}")
                    for t in range(NS_)
                ]

            def kloop_range(ot, pots, k0, k1):
                for k in range(k0, k1):
                    for t in range(NS_):
                        nc.tensor.matmul(
                            pots[t][:],
                            w_tiles[ot][:, k, :],
                            x_sb[:, k, t * NSLAB : (t + 1) * NSLAB],
                            start=(k == 0),
                            stop=False,
                        )

            def base_kloop(ot):
                if ot not in w_tiles:
                    load_w(ot)
                pots = alloc_pots()
                kloop_range(ot, pots, 0, KT)
                return pots

            def base_tail(ot, pots, chunks=1):
                # per slab: finish the accumulation (lora+bias row), copy the
                # PSUM bank out, and DMA that half immediately — keeps the
                # end-of-kernel drain to half an o-tile instead of a full one.
                # The very last o-tile drains in quarter-slab chunks so the
                # final copy+DMA exposure after the last matmul is minimal.
                osl = slice(ot * P, (ot + 1) * P)
                o_sb = opool.tile([P, TPC], F32, tag="o", name="o_sb")
                for t in range(NS_):
                    tsl = slice(t * NSLAB, (t + 1) * NSLAB)
                    rb = t * ROWB
                    # slab 0 in PE rows 0..32, slab 1 in rows 64..96: disjoint
                    # row groups, so the two 512-cycle matmuls overlap
                    nc.tensor.matmul(
                        pots[t][:],
                        bt_sb[rb : rb + ERA, osl],
                        wp_sb[rb : rb + ERA, tsl],
                        start=False,
                        stop=True,
                    )
                    for c in range(chunks):
                        w = NSLAB // chunks
                        csl = slice(t * NSLAB + c * w, t * NSLAB + (c + 1) * w)
                        psl = slice(c * w, (c + 1) * w)
                        # alternate copy engines so slab 1's copies don't
                        # queue behind slab 0's on the DVE
                        if t == 0:
                            nc.vector.tensor_copy(o_sb[:, csl], pots[t][:, psl])
                        else:
                            nc.scalar.copy(o_sb[:, csl], pots[t][:, psl])
                        nc.sync.dma_start(out=od[osl, csl], in_=o_sb[:, csl])
                del w_tiles[ot]

            for rep in range(reps):
                if rep == 0:
                    # k-interleaved startup: the proj/router and o-tile-0
                    # matmuls share each x k-tile, so the PE tracks the x DMA
                    # stream instead of running dry; o-tile 1's k-loop follows
                    # un-interleaved to cover the gating chain's ACT/DVE
                    # latency before the o-tile-0/1 lora tails need wp_sb.
                    pots0 = [
                        psum_base.tile([P, NSLAB], F32, tag=f"po{t}", name=f"po{t}")
                        for t in range(NS_)
                    ]
                    LAG = 6  # o-tile 0's k-index trails proj's so its
                    # first matmul reaches the PE only after W0's DMA lands
                    for k in range(KT + LAG):
                        if k < KT:
                            for t in range(NS_):
                                # slab 0 -> array cols 0..35, slab 1 -> 64..99
                                nc.tensor.matmul(
                                    pps[t][:],
                                    art_sb[:, k, :],
                                    x_sb[:, k, t * NSLAB : (t + 1) * NSLAB],
                                    start=(k == 0),
                                    stop=(k == KT - 1),
                                )
                        if k >= LAG:
                            kb = k - LAG
                            for t in range(NS_):
                                nc.tensor.matmul(
                                    pots0[t][:],
                                    w_tiles[0][:, kb, :],
                                    x_sb[:, kb, t * NSLAB : (t + 1) * NSLAB],
                                    start=(kb == 0),
                                    stop=False,
                                )
                    # prefetch the next W tiles NOW: later in the DMA queue
                    # they'd sit behind the o-tile-0/1 output DMAs, which
                    # can't start until their PSUM copies (~45us in)
                    load_w(2)
                    load_w(3)
                    # the shared mini-matmul bank (see gating comment above)
                    gat_ps = psum_proj.tile(
                        [ROWB + 32 + E, NSLAB], F32, tag="gat", name="gat_ps"
                    )
                    for t in range(NS_):
                        gat_exp(t)
                    # o-tile 1's k-loop in slices between gating stages: each
                    # PE->DVE->PE hop of the softmax chain hides under ~1.7us
                    # of base matmuls instead of stalling the in-order PE
                    pots1 = alloc_pots()
                    kloop_range(1, pots1, 0, 8)
                    for t in range(NS_):
                        gat_sum(t, gat_ps)
                    for t in range(NS_):
                        gat_recip(t)
                    kloop_range(1, pots1, 8, 16)
                    for t in range(NS_):
                        gat_r4(t, gat_ps)
                    for t in range(NS_):
                        gat_g4(t)
                    kloop_range(1, pots1, 16, 24)
                    for t in range(NS_):
                        gat_g32(t, gat_ps)
                    for t in range(NS_):
                        gat_wp(t)
                    kloop_range(1, pots1, 24, KT)
                    base_tail(0, pots0)
                    base_tail(1, pots1)
                    start_ot = 2
                else:
                    start_ot = 0
                for ot in range(start_ot, OT):
                    # keep W two o-tiles ahead of the PE (nearest-first)
                    for o2 in (ot, ot + 1, ot + 2):
                        if o2 < OT and o2 not in w_tiles:
                            load_w(o2)
                    pots = base_kloop(ot)
                    base_tail(ot, pots)

    nc.compile()
    if DEDUPE_LDW:
        _dedupe_ldweights(nc)
    return nc


def get_nc():
    global _NC
    if _NC is None:
        _NC = build_nc()
    return _NC


def _prep_shared(W, b, A, Bm, Wr):
    # W (OUT, IN) -> wd[ot, p, k, o] = W[ot*128+o, k*128+p], fp16, contiguous
    wd = np.ascontiguousarray(
        W.reshape(OT, P, KT, P).transpose(0, 3, 2, 1).astype(np.float16)
    )
    # [A (E,R,IN) flattened; Wr (E,IN)] -> art[p, k, j] = AR[j, k*128+p]
    ar = np.concatenate([A.reshape(ER, IN), Wr], axis=0)  # (36, IN)
    artd = np.ascontiguousarray(
        ar.T.reshape(KT, P, ER + E).transpose(1, 0, 2).astype(np.float16)
    )
    # Bcat rows er = Bm[e,:,r]; row 32 = bias. Duplicated at row offset 64 so
    # the two token slabs' lora matmuls can use disjoint PE row groups.
    bt = np.concatenate([Bm.transpose(0, 2, 1).reshape(ER, OUT), b[None, :]], axis=0)
    btd = np.zeros((ROWB + ERA, OUT), np.float16)
    btd[0:ERA] = bt.astype(np.float16)
    btd[ROWB : ROWB + ERA] = btd[0:ERA]
    sel = np.zeros((E, ER), np.float16)
    for e in range(E):
        sel[e, e * R : (e + 1) * R] = SCALE
    return wd, artd, btd, sel


def _prep_x_shard(xt, c):
    xs = xt[c * TPC : (c + 1) * TPC]  # (TPC, IN)
    return np.ascontiguousarray(
        xs.T.reshape(KT, P, TPC).transpose(1, 0, 2).astype(np.float16)
    )


def make_in_maps(x, W, b, A, Bm, Wr):
    xt = np.asarray(x, np.float32).reshape(TOK, IN)
    wd, artd, btd, sel = _prep_shared(
        np.asarray(W, np.float32),
        np.asarray(b, np.float32),
        np.asarray(A, np.float32),
        np.asarray(Bm, np.float32),
        np.asarray(Wr, np.float32),
    )
    return [
        {
            "xd": _prep_x_shard(xt, c),
            "wd": wd,
            "artd": artd,
            "btd": btd,
            "seld": sel,
        }
        for c in range(N_CORES)
    ]


def gather_out(results):
    # per-core od is (OUT, TPC); tokens are sharded contiguously
    return np.concatenate([r["od"].T for r in results], axis=0).reshape(B, S, OUT)


def kernel(x, W, b, A, Bm, Wr, _trace=False):
    nc = get_nc()
    in_maps = make_in_maps(x, W, b, A, Bm, Wr)
    res = bass_utils.run_bass_kernel_spmd(
        nc, in_maps, core_ids=list(range(N_CORES)), trace=_trace
    )
    out = gather_out(res.results)
    if _trace:
        return out, res
    return out



# revision 55
# speedup vs baseline: 1.2544x; 1.2544x over previous
"""MoE QLoRA linear kernel for Trainium2 (8 NeuronCores, data-parallel over tokens).

Computes, for x:(B,S,IN) f32:
    base  = x @ W.T + b
    gates = softmax(x @ Wr.T)                       # (tok, E)
    proj  = x @ A[e].T                              # (tok, E, R)
    out   = base + sum_e SCALE * gates[...,e] * (proj[...,e,:] @ Bm[e].T)

Key algebraic fold: the gated expert mix is a single rank-(E*R) matmul:
    wproj[t, er] = SCALE * gates[t, e] * proj[t, er]          (er = e*R+r)
    lora[t, o]   = sum_er wproj[t, er] * Bcat[er, o]          (Bcat[er,o] = Bm[e,o,r])
and the bias b is folded in as an extra contraction row (wproj row of ones,
Bcat row = b), so base+lora+bias all accumulate in one PSUM group on the PE.

Per-core kernel (1024 tokens), everything oriented (feature-partition, token-free):
  phase 1: PSUM(36,512) = [A;Wr]^T-stationary matmuls over 32 k-tiles ->
           proj rows 0..31, router logits rows 32..35 (col-tiled: the two
           token slabs stream concurrently); softmax via exp + staged PE
           ones-matmul partition reductions/broadcasts; wproj written fp16.
           The o-tile-0 base k-loop is interleaved into the proj k-loop
           (k-index staggered behind it) so the PE tracks the x DMA stream,
           and o-tile 1's k-loop is sliced between the gating stages.
  phase 2: for each of 32 o-tiles: out(128o, t) = W-tile-stationary matmul
           over 32 k-tiles + one lora matmul (k=33) accumulated into PSUM,
           copy to SBUF, DMA out as (OUT, tok); host transposes back.

All matmul inputs are fp16 (host-cast; PE runs fp16 at full bf16 rate,
fp32 PSUM accumulation). Host pre-tiles all layouts so every DMA is
contiguous and the kernel needs zero on-chip transposes.

Optimizations landed on top of the first working version (600us/core):
 - LDWEIGHTS dedupe (_dedupe_ldweights): walrus runs --enable-ldw-opt=false
   (=true crashes its codegen on our standalone LDWs), so each matmul gets
   its own LDWEIGHTS; the second load of each same-stationary (o-tile, k)
   pair is redundant. Deleting them post-compile is hardware-validated and
   worth ~30us/rep (probe-measured). The signature includes tile_position/
   tile_size, and any self-loading (fp32) matmul resets it — without that,
   two deletions with an interleaved gating matmul corrupted o-tile 1.
 - fp16 gating matmuls (fp32 ran at 1/4 PE rate): ~4us.
 - Row-tiled lora tails: slab 0 in PE rows 0..32, slab 1 in rows 64..96
   (bt/wp duplicated at partition offset 64), so the two 512-cycle lora
   matmuls stream concurrently: ~7us.
 - Col-tiled phase-1 proj: slab 0 -> PSUM partitions 0..35 (PE col group 0),
   slab 1 -> partitions 64..99 of its own bank (col group 64), sharing the
   art stationary: ~7us.
 - DMA issue order tracks PE consumption (art k0-3, x k0, W0, ... W two
   o-tiles ahead of the PE), and the startup k-loop interleaves proj with
   o-tile 0 so the PE follows the x DMA stream: ~6us of startup idle.
 - Staged gating: the softmax chain is a serial PE->DVE->PE ping-pong, so
   its stages are emitted with slices of o-tile 1's k-loop between them
   (every cross-engine hop hides under ~1.7us of base matmuls), and all six
   mini-matmul outputs share one manually-sliced PSUM bank at 32-aligned
   partition offsets (single-shot matmuls: a later start's whole-bank
   has_written clear resets accumulate semantics, never stored data): ~4us.

Measured device behavior (pure-matmul probe, ldw_probe.py): the chip
oscillates between ~2.4GHz and ~2.0GHz PE power states with ~0.1-1s dwell;
per-rep slope is ~446us fast / ~528us slow with dedupe (the fp16 streaming
roofline for 2112 N=512 matmuls is 450us at 2.4GHz, i.e. LDWEIGHTS is fully
hidden after dedupe). test.py therefore reports the median per-rep slope
over many short alternating loops, scaled by the cost-model full/base ratio.
"""

import numpy as np

import concourse.bass as bass
import concourse.tile as tile
from concourse import bacc, mybir
from concourse import bass_utils

# Problem shape (hardcoded; kernel.py must be self-contained)
B, S, IN, OUT, E, R = 4, 2048, 4096, 4096, 4, 8
SCALE = 16.0 / 8.0
N_CORES = 8
TOK = B * S                  # 8192 tokens
TPC = TOK // N_CORES         # 1024 tokens per core
P = 128                      # partitions
KT = IN // P                 # 32 k-tiles (contraction)
OT = OUT // P                # 32 output tiles
NSLAB = 512                  # moving-operand free size (PSUM bank = 512 f32)
NS = TPC // NSLAB            # 2 token slabs per core
ER = E * R                   # 32 low-rank rows
ERA = ER + 1                 # +1 ones row (bias fold)
ROWB = 64                    # partition offset of slab-1's lora row-tile

F16 = mybir.dt.float16
F32 = mybir.dt.float32

_NC = None

# Post-compile pass: delete redundant consecutive InstLdweights from the PE
# stream. Walrus (--enable-ldw-opt=false) emits one LDWEIGHTS per matmul;
# when consecutive matmuls share the same stationary tile (the two token
# slabs of each (o-tile, k)), the second load is identical, wait/update-free,
# and costs ~53ns of serialized PE time. Deleting it lets the following
# non-self-loading matmul reuse the already-loaded weights.
DEDUPE_LDW = True


def _dedupe_ldweights(nc):
    removed = 0
    for blk in nc.m.functions[0].blocks:
        keep = []
        prev_sig = None
        for inst in blk.instructions:
            if isinstance(inst, mybir.InstLdweights):
                sig = (
                    str(inst.ins[0]),
                    str(inst.tile_position),
                    str(inst.tile_size),
                    str(inst.perf_mode),
                    str(inst.is_transpose),
                )
                if sig == prev_sig and not inst.has_wait() and not inst.has_update():
                    removed += 1
                    continue
                prev_sig = sig
            elif isinstance(inst, mybir.InstMatmult) and inst.ldweights is not False:
                # self-loading matmul (fp32 gating) reloads the PE array's
                # stationary weights — following matmuls need a fresh LDW
                prev_sig = None
            keep.append(inst)
        blk.instructions = keep
    return removed


def build_nc(reps=1, ns=NS):
    NS_ = ns
    nc = bacc.Bacc("TRN2", target_bir_lowering=False, debug=False)

    xd = nc.dram_tensor("xd", [P, KT, TPC], F16, kind="ExternalInput")
    wd = nc.dram_tensor("wd", [OT, P, KT, P], F16, kind="ExternalInput")
    artd = nc.dram_tensor("artd", [P, KT, ER + E], F16, kind="ExternalInput")
    # btd rows 0..32 = [Bcat; bias]; rows 64..96 = same again, so the two
    # slabs' lora matmuls can run concurrently as PE row-tiles (0-63 / 64-127)
    btd = nc.dram_tensor("btd", [ROWB + ERA, OUT], F16, kind="ExternalInput")
    seld = nc.dram_tensor("seld", [E, ER], F16, kind="ExternalInput")
    od = nc.dram_tensor("od", [OUT, TPC], F32, kind="ExternalOutput")

    with tile.TileContext(nc) as tc:
        with (
            tc.tile_pool(name="consts", bufs=1) as consts,
            tc.tile_pool(name="wpool", bufs=4) as wpool,
            tc.tile_pool(name="opool", bufs=3) as opool,
            tc.tile_pool(name="small", bufs=2) as small,
            tc.tile_pool(name="psum_proj", bufs=1, space="PSUM") as psum_proj,
            tc.tile_pool(name="psum_base", bufs=2, space="PSUM") as psum_base,
        ):
            # DMA issue order tracks the PE consumption schedule: phase 1
            # consumes art k-tiles + x k-tiles in order from t=0; W tile 0
            # isn't needed until the o-tile-0 k-loop (~15us in), W tile 1 and
            # the bias/sel tiles later still. Keeping the big W loads out of
            # the head of the queue cuts the PE's startup DMA wait.
            art_sb = consts.tile([P, KT, ER + E], F16)
            nc.sync.dma_start(out=art_sb[:, 0:4, :], in_=artd[:, 0:4, :])

            w_tiles = {}

            def load_w(ot):
                w_sb = wpool.tile([P, KT, P], F16, tag="w", name="w_sb")
                nc.sync.dma_start(out=w_sb[:], in_=wd[ot])
                w_tiles[ot] = w_sb

            # Resident activations: x^T tiled (p=i%128, k=i//128, t), fp16, 8 MiB.
            # W0 right after x k0: the k-interleaved startup loop consumes
            # W0's k-tiles from its 4th matmul on.
            x_sb = consts.tile([P, KT, TPC], F16)
            nc.sync.dma_start(out=x_sb[:, 0, :], in_=xd[:, 0, :])
            load_w(0)
            nc.sync.dma_start(out=art_sb[:, 4:, :], in_=artd[:, 4:, :])
            for k in range(1, 20):
                nc.sync.dma_start(out=x_sb[:, k, :], in_=xd[:, k, :])
            # W1/sel/bt aren't consumed until ~30us/~45us in; keeping them
            # behind the first 20 x k-tiles keeps the startup loop x-fed
            load_w(1)
            sel_sb = consts.tile([E, ER], F16)
            nc.sync.dma_start(out=sel_sb[:], in_=seld[:])
            for k in range(20, 25):
                nc.sync.dma_start(out=x_sb[:, k, :], in_=xd[:, k, :])
            bt_sb = consts.tile([ROWB + ERA, OUT], F16)
            nc.sync.dma_start(out=bt_sb[:], in_=btd[:])
            for k in range(25, KT):
                nc.sync.dma_start(out=x_sb[:, k, :], in_=xd[:, k, :])

            ones_e1 = consts.tile([E, 1], F16)
            nc.vector.memset(ones_e1[:], 1.0)
            ones_1e = consts.tile([1, E], F16)
            nc.vector.memset(ones_1e[:], 1.0)
            # Gated low-rank projection, fp16. Slab 0 lives in rows 0..31
            # (+ones row 32), slab 1 in rows 64..95 (+ones row 96) so the two
            # lora matmuls occupy disjoint PE row groups and run concurrently.
            wp_sb = consts.tile([ROWB + ERA, TPC], F16)
            nc.vector.memset(wp_sb[ER : ER + 1, :], 1.0)
            nc.vector.memset(wp_sb[ROWB + ER : ROWB + ERA, :], 1.0)

            # ---------- phase 1: proj + router matmul PSUM tiles ----------
            # Slab 0's proj output sits at PSUM partitions 0..35 (PE col group
            # 0), slab 1's at partitions 64..99 of its own bank (col group
            # 64): the two matmuls share the art stationary but occupy
            # disjoint PE column groups, so they stream concurrently.
            # Per slab: rows +0..31 = proj^T (er), rows +32..35 router logits.
            pp_tiles = [
                psum_proj.tile(
                    [t * ROWB + ER + E, NSLAB], F32, tag=f"pp{t}", name=f"pp{t}"
                )
                for t in range(NS_)
            ]
            pps = [pp_tiles[t][t * ROWB : t * ROWB + ER + E, :] for t in range(NS_)]

            # ---------- gating: softmax over the 4 expert rows ----------
            # (no max-sub: |logit| < ~8). Partition reductions/broadcasts are
            # tiny PE matmuls (fp16 operands: fp32 ran at 1/4 rate). The chain
            # is a serial PE->DVE->PE ping-pong (sum -> recip -> bcast -> mul
            # -> select), so it is emitted in STAGES with slices of o-tile 1's
            # k-loop between them: every cross-engine hop hides under ~1.7us
            # of base matmuls instead of stalling the in-order PE.
            #
            # All six mini-matmul outputs share ONE manually-sliced PSUM bank
            # at 32-aligned partition offsets (slab 0 at 0..35, slab 1 at
            # 64..99 -> disjoint PE col groups, concurrent). This is safe
            # because each mini-matmul is single-shot (start+stop): a later
            # start's whole-bank has_written clear resets accumulate semantics
            # only, never stored data, and the one region overlap (g32 over
            # s) is ordered by the true dependency chain through recip.
            ga = [dict() for _ in range(NS_)]

            def gat_exp(t):
                g = ga[t]
                g["e"] = small.tile([E, NSLAB], F16, tag="e", name="e_sb")
                nc.scalar.activation(
                    g["e"][:],
                    pps[t][ER : ER + E, :],
                    mybir.ActivationFunctionType.Exp,
                )

            def gat_sum(t, gat_ps):
                g = ga[t]
                g["s"] = gat_ps[t * ROWB : t * ROWB + 1, :]
                nc.tensor.matmul(g["s"], ones_e1[:], g["e"][:])  # sum_e exp

            def gat_recip(t):
                g = ga[t]
                g["r"] = small.tile([1, NSLAB], F16, tag="r", name="r_sb")
                with nc.allow_low_precision(
                    reason="softmax gates are O(1) and scale only the small "
                    "lora term; fp16 is ample"
                ):
                    nc.vector.reciprocal(g["r"][:], g["s"])

            def gat_r4(t, gat_ps):
                g = ga[t]
                g["r4"] = gat_ps[t * ROWB + 32 : t * ROWB + 32 + E, :]
                # bcast to 4 rows; explicit tile_position (auto-derivation
                # rejects output base partition 96)
                nc.tensor.matmul(
                    g["r4"],
                    ones_1e[:],
                    g["r"][:],
                    tile_position=(0, t * ROWB + 32),
                )

            def gat_g4(t):
                g = ga[t]
                g["g4"] = small.tile([E, NSLAB], F16, tag="g4", name="g4_sb")
                nc.vector.tensor_mul(g["g4"][:], g["e"][:], g["r4"])

            def gat_g32(t, gat_ps):
                # (SCALE * gate)[er, t] via 0/1*SCALE selection matmul
                g = ga[t]
                g["g32"] = gat_ps[t * ROWB : t * ROWB + ER, :]
                nc.tensor.matmul(g["g32"], sel_sb[:], g["g4"][:])

            def gat_wp(t):
                g = ga[t]
                tsl = slice(t * NSLAB, (t + 1) * NSLAB)
                rb = t * ROWB  # slab-1 wproj lives at partition offset 64
                # walrus: tensor_tensor may read at most one operand from PSUM
                g32_sb = small.tile([ER, NSLAB], F32, tag="g32s", name="g32_sb")
                nc.vector.tensor_copy(g32_sb[:], g["g32"])
                nc.vector.tensor_mul(
                    wp_sb[rb : rb + ER, tsl], pps[t][0:ER, :], g32_sb[:]
                )

            # ---------- phase 2: base matmul + lora + bias ----------
            def alloc_pots():
                return [
                    psum_base.tile([P, NSLAB], F32, tag=f"po{t}", name=f"po{t}")
                    for t in range(NS_)
                ]

            def kloop_range(ot, pots, k0, k1):
                for k in range(k0, k1):
                    for t in range(NS_):
                        nc.tensor.matmul(
                            pots[t][:],
                            w_tiles[ot][:, k, :],
                            x_sb[:, k, t * NSLAB : (t + 1) * NSLAB],
                            start=(k == 0),
                            stop=False,
                        )

            def base_kloop(ot):
                if ot not in w_tiles:
                    load_w(ot)
                pots = alloc_pots()
                kloop_range(ot, pots, 0, KT)
                return pots

            def base_tail(ot, pots, chunks=1):
                # per slab: finish the accumulation (lora+bias row), copy the
                # PSUM bank out, and DMA that half immediately — keeps the
                # end-of-kernel drain to half an o-tile instead of a full one.
                # The very last o-tile drains in quarter-slab chunks so the
                # final copy+DMA exposure after the last matmul is minimal.
                osl = slice(ot * P, (ot + 1) * P)
                o_sb = opool.tile([P, TPC], F32, tag="o", name="o_sb")
                for t in range(NS_):
                    tsl = slice(t * NSLAB, (t + 1) * NSLAB)
                    rb = t * ROWB
                    # slab 0 in PE rows 0..32, slab 1 in rows 64..96: disjoint
                    # row groups, so the two 512-cycle matmuls overlap
                    nc.tensor.matmul(
                        pots[t][:],
                        bt_sb[rb : rb + ERA, osl],
                        wp_sb[rb : rb + ERA, tsl],
                        start=False,
                        stop=True,
                    )
                    for c in range(chunks):
                        w = NSLAB // chunks
                        csl = slice(t * NSLAB + c * w, t * NSLAB + (c + 1) * w)
                        psl = slice(c * w, (c + 1) * w)
                        # alternate copy engines so slab 1's copies don't
                        # queue behind slab 0's on the DVE
                        if t == 0:
                            nc.vector.tensor_copy(o_sb[:, csl], pots[t][:, psl])
                        else:
                            nc.scalar.copy(o_sb[:, csl], pots[t][:, psl])
                        nc.sync.dma_start(out=od[osl, csl], in_=o_sb[:, csl])
                del w_tiles[ot]

            for rep in range(reps):
                if rep == 0:
                    # k-interleaved startup: the proj/router and o-tile-0
                    # matmuls share each x k-tile, so the PE tracks the x DMA
                    # stream instead of running dry; o-tile 1's k-loop follows
                    # un-interleaved to cover the gating chain's ACT/DVE
                    # latency before the o-tile-0/1 lora tails need wp_sb.
                    pots0 = [
                        psum_base.tile([P, NSLAB], F32, tag=f"po{t}", name=f"po{t}")
                        for t in range(NS_)
                    ]
                    LAG = 6  # o-tile 0's k-index trails proj's so its
                    # first matmul reaches the PE only after W0's DMA lands
                    for k in range(KT + LAG):
                        if k < KT:
                            for t in range(NS_):
                                # slab 0 -> array cols 0..35, slab 1 -> 64..99
                                nc.tensor.matmul(
                                    pps[t][:],
                                    art_sb[:, k, :],
                                    x_sb[:, k, t * NSLAB : (t + 1) * NSLAB],
                                    start=(k == 0),
                                    stop=(k == KT - 1),
                                )
                        if k >= LAG:
                            kb = k - LAG
                            for t in range(NS_):
                                nc.tensor.matmul(
                                    pots0[t][:],
                                    w_tiles[0][:, kb, :],
                                    x_sb[:, kb, t * NSLAB : (t + 1) * NSLAB],
                                    start=(kb == 0),
                                    stop=False,
                                )
                    # prefetch the next W tiles NOW: later in the DMA queue
                    # they'd sit behind the o-tile-0/1 output DMAs, which
                    # can't start until their PSUM copies (~45us in)
                    load_w(2)
                    load_w(3)
                    # the shared mini-matmul bank (see gating comment above)
                    gat_ps = psum_proj.tile(
                        [ROWB + 32 + E, NSLAB], F32, tag="gat", name="gat_ps"
                    )
                    for t in range(NS_):
                        gat_exp(t)
                    # o-tile 1's k-loop in slices between gating stages: each
                    # PE->DVE->PE hop of the softmax chain hides under ~1.7us
                    # of base matmuls instead of stalling the in-order PE
                    pots1 = alloc_pots()
                    kloop_range(1, pots1, 0, 8)
                    for t in range(NS_):
                        gat_sum(t, gat_ps)
                    for t in range(NS_):
                        gat_recip(t)
                    kloop_range(1, pots1, 8, 16)
                    for t in range(NS_):
                        gat_r4(t, gat_ps)
                    for t in range(NS_):
                        gat_g4(t)
                    kloop_range(1, pots1, 16, 24)
                    for t in range(NS_):
                        gat_g32(t, gat_ps)
                    for t in range(NS_):
                        gat_wp(t)
                    kloop_range(1, pots1, 24, KT)
                    base_tail(0, pots0)
                    base_tail(1, pots1)
                    start_ot = 2
                else:
                    start_ot = 0
                for ot in range(start_ot, OT):
                    # keep W two o-tiles ahead of the PE (nearest-first)
                    for o2 in (ot, ot + 1, ot + 2):
                        if o2 < OT and o2 not in w_tiles:
                            load_w(o2)
                    pots = base_kloop(ot)
                    base_tail(ot, pots)

    nc.compile()
    if DEDUPE_LDW:
        _dedupe_ldweights(nc)
    return nc


def get_nc():
    global _NC
    if _NC is None:
        _NC = build_nc()
    return _NC


def _prep_shared(W, b, A, Bm, Wr):
    # W (OUT, IN) -> wd[ot, p, k, o] = W[ot*128+o, k*128+p], fp16, contiguous
    wd = np.ascontiguousarray(
        W.reshape(OT, P, KT, P).transpose(0, 3, 2, 1).astype(np.float16)
    )
    # [A (E,R,IN) flattened; Wr (E,IN)] -> art[p, k, j] = AR[j, k*128+p]
    ar = np.concatenate([A.reshape(ER, IN), Wr], axis=0)  # (36, IN)
    artd = np.ascontiguousarray(
        ar.T.reshape(KT, P, ER + E).transpose(1, 0, 2).astype(np.float16)
    )
    # Bcat rows er = Bm[e,:,r]; row 32 = bias. Duplicated at row offset 64 so
    # the two token slabs' lora matmuls can use disjoint PE row groups.
    bt = np.concatenate([Bm.transpose(0, 2, 1).reshape(ER, OUT), b[None, :]], axis=0)
    btd = np.zeros((ROWB + ERA, OUT), np.float16)
    btd[0:ERA] = bt.astype(np.float16)
    btd[ROWB : ROWB + ERA] = btd[0:ERA]
    sel = np.zeros((E, ER), np.float16)
    for e in range(E):
        sel[e, e * R : (e + 1) * R] = SCALE
    return wd, artd, btd, sel


def _prep_x_shard(xt, c):
    xs = xt[c * TPC : (c + 1) * TPC]  # (TPC, IN)
    return np.ascontiguousarray(
        xs.T.reshape(KT, P, TPC).transpose(1, 0, 2).astype(np.float16)
    )


def make_in_maps(x, W, b, A, Bm, Wr):
    xt = np.asarray(x, np.float32).reshape(TOK, IN)
    wd, artd, btd, sel = _prep_shared(
        np.asarray(W, np.float32),
        np.asarray(b, np.float32),
        np.asarray(A, np.float32),
        np.asarray(Bm, np.float32),
        np.asarray(Wr, np.float32),
    )
    return [
        {
            "xd": _prep_x_shard(xt, c),
            "wd": wd,
            "artd": artd,
            "btd": btd,
            "seld": sel,
        }
        for c in range(N_CORES)
    ]


def gather_out(results):
    # per-core od is (OUT, TPC); tokens are sharded contiguously
    return np.concatenate([r["od"].T for r in results], axis=0).reshape(B, S, OUT)


def kernel(x, W, b, A, Bm, Wr, _trace=False):
    nc = get_nc()
    in_maps = make_in_maps(x, W, b, A, Bm, Wr)
    res = bass_utils.run_bass_kernel_spmd(
        nc, in_maps, core_ids=list(range(N_CORES)), trace=_trace
    )
    out = gather_out(res.results)
    if _trace:
        return out, res
    return out



# revision 57
# speedup vs baseline: 1.3537x; 1.0792x over previous
"""MoE QLoRA linear kernel for Trainium2 (8 NeuronCores, data-parallel over tokens).

Computes, for x:(B,S,IN) f32:
    base  = x @ W.T + b
    gates = softmax(x @ Wr.T)                       # (tok, E)
    proj  = x @ A[e].T                              # (tok, E, R)
    out   = base + sum_e SCALE * gates[...,e] * (proj[...,e,:] @ Bm[e].T)

Key algebraic fold: the gated expert mix is a single rank-(E*R) matmul:
    wproj[t, er] = SCALE * gates[t, e] * proj[t, er]          (er = e*R+r)
    lora[t, o]   = sum_er wproj[t, er] * Bcat[er, o]          (Bcat[er,o] = Bm[e,o,r])
and the bias b is folded in as an extra contraction row (wproj row of ones,
Bcat row = b), so base+lora+bias all accumulate in one PSUM group on the PE.

Per-core kernel (1024 tokens), everything oriented (feature-partition, token-free):
  phase 1: PSUM(36,512) = [A;Wr]^T-stationary matmuls over 32 k-tiles ->
           proj rows 0..31, router logits rows 32..35 (col-tiled: the two
           token slabs stream concurrently); softmax via exp + staged PE
           ones-matmul partition reductions/broadcasts; wproj written fp16.
           The o-tile-0 base k-loop is interleaved into the proj k-loop
           (k-index staggered behind it) so the PE tracks the x DMA stream,
           and o-tile 1's k-loop is sliced between the gating stages.
  phase 2: for each of 32 o-tiles: out(128o, t) = W-tile-stationary matmul
           over 32 k-tiles + one lora matmul (k=33) accumulated into PSUM,
           copy to SBUF, DMA out as (OUT, tok); host transposes back.

All matmul inputs are fp16 (host-cast; PE runs fp16 at full bf16 rate,
fp32 PSUM accumulation). Host pre-tiles all layouts so every DMA is
contiguous and the kernel needs zero on-chip transposes.

Optimizations landed on top of the first working version (600us/core):
 - LDWEIGHTS dedupe (_dedupe_ldweights): walrus runs --enable-ldw-opt=false
   (=true crashes its codegen on our standalone LDWs), so each matmul gets
   its own LDWEIGHTS; the second load of each same-stationary (o-tile, k)
   pair is redundant. Deleting them post-compile is hardware-validated and
   worth ~30us/rep (probe-measured). The signature includes tile_position/
   tile_size, and any self-loading (fp32) matmul resets it — without that,
   two deletions with an interleaved gating matmul corrupted o-tile 1.
 - fp16 gating matmuls (fp32 ran at 1/4 PE rate): ~4us.
 - Row-tiled lora tails: slab 0 in PE rows 0..32, slab 1 in rows 64..96
   (bt/wp duplicated at partition offset 64), so the two 512-cycle lora
   matmuls stream concurrently: ~7us.
 - Col-tiled phase-1 proj: slab 0 -> PSUM partitions 0..35 (PE col group 0),
   slab 1 -> partitions 64..99 of its own bank (col group 64), sharing the
   art stationary: ~7us.
 - DMA issue order tracks PE consumption (art k0-3, x k0, W0, ... W two
   o-tiles ahead of the PE), and the startup k-loop interleaves proj with
   o-tile 0 so the PE follows the x DMA stream: ~6us of startup idle.
 - Staged gating: the softmax chain is a serial PE->DVE->PE ping-pong, so
   its stages are emitted with slices of o-tile 1's k-loop between them
   (every cross-engine hop hides under ~1.7us of base matmuls), and all six
   mini-matmul outputs share one manually-sliced PSUM bank at 32-aligned
   partition offsets (single-shot matmuls: a later start's whole-bank
   has_written clear resets accumulate semantics, never stored data): ~4us.

Measured device behavior (pure-matmul probe, ldw_probe.py): the chip
oscillates between ~2.4GHz and ~2.0GHz PE power states with ~0.1-1s dwell;
per-rep slope is ~446us fast / ~528us slow with dedupe (the fp16 streaming
roofline for 2112 N=512 matmuls is 450us at 2.4GHz, i.e. LDWEIGHTS is fully
hidden after dedupe). test.py therefore reports the median per-rep slope
over many short alternating loops, scaled by the cost-model full/base ratio.
"""

import numpy as np

import concourse.bass as bass
import concourse.tile as tile
from concourse import bacc, mybir
from concourse import bass_utils

# Problem shape (hardcoded; kernel.py must be self-contained)
B, S, IN, OUT, E, R = 4, 2048, 4096, 4096, 4, 8
SCALE = 16.0 / 8.0
N_CORES = 8
TOK = B * S                  # 8192 tokens
TPC = TOK // N_CORES         # 1024 tokens per core
P = 128                      # partitions
KT = IN // P                 # 32 k-tiles (contraction)
OT = OUT // P                # 32 output tiles
NSLAB = 512                  # moving-operand free size (PSUM bank = 512 f32)
NS = TPC // NSLAB            # 2 token slabs per core
ER = E * R                   # 32 low-rank rows
ERA = ER + 1                 # +1 ones row (bias fold)
ROWB = 64                    # partition offset of slab-1's lora row-tile

F16 = mybir.dt.float16
F32 = mybir.dt.float32

_NC = None

# Post-compile pass: delete redundant consecutive InstLdweights from the PE
# stream. Walrus (--enable-ldw-opt=false) emits one LDWEIGHTS per matmul;
# when consecutive matmuls share the same stationary tile (the two token
# slabs of each (o-tile, k)), the second load is identical, wait/update-free,
# and costs ~53ns of serialized PE time. Deleting it lets the following
# non-self-loading matmul reuse the already-loaded weights.
DEDUPE_LDW = True


def _dedupe_ldweights(nc):
    removed = 0
    for blk in nc.m.functions[0].blocks:
        keep = []
        prev_sig = None
        for inst in blk.instructions:
            if isinstance(inst, mybir.InstLdweights):
                sig = (
                    str(inst.ins[0]),
                    str(inst.tile_position),
                    str(inst.tile_size),
                    str(inst.perf_mode),
                    str(inst.is_transpose),
                )
                if sig == prev_sig and not inst.has_wait() and not inst.has_update():
                    removed += 1
                    continue
                prev_sig = sig
            elif isinstance(inst, mybir.InstMatmult) and inst.ldweights is not False:
                # self-loading matmul (fp32 gating) reloads the PE array's
                # stationary weights — following matmuls need a fresh LDW
                prev_sig = None
            keep.append(inst)
        blk.instructions = keep
    return removed


def build_nc(reps=1, ns=NS):
    NS_ = ns
    nc = bacc.Bacc("TRN2", target_bir_lowering=False, debug=False)

    xd = nc.dram_tensor("xd", [P, KT, TPC], F16, kind="ExternalInput")
    wd = nc.dram_tensor("wd", [OT, P, KT, P], F16, kind="ExternalInput")
    artd = nc.dram_tensor("artd", [P, KT, ER + E], F16, kind="ExternalInput")
    # btd rows 0..32 = [Bcat; bias]; rows 64..96 = same again, so the two
    # slabs' lora matmuls can run concurrently as PE row-tiles (0-63 / 64-127)
    btd = nc.dram_tensor("btd", [ROWB + ERA, OUT], F16, kind="ExternalInput")
    seld = nc.dram_tensor("seld", [E, ER], F16, kind="ExternalInput")
    od = nc.dram_tensor("od", [OUT, TPC], F32, kind="ExternalOutput")

    with tile.TileContext(nc) as tc:
        with (
            tc.tile_pool(name="consts", bufs=1) as consts,
            tc.tile_pool(name="wpool", bufs=4) as wpool,
            tc.tile_pool(name="opool", bufs=3) as opool,
            tc.tile_pool(name="small", bufs=2) as small,
            tc.tile_pool(name="psum_proj", bufs=1, space="PSUM") as psum_proj,
            tc.tile_pool(name="psum_base", bufs=2, space="PSUM") as psum_base,
        ):
            # DMA issue order tracks the PE consumption schedule: phase 1
            # consumes art k-tiles + x k-tiles in order from t=0; W tile 0
            # isn't needed until the o-tile-0 k-loop (~15us in), W tile 1 and
            # the bias/sel tiles later still. Keeping the big W loads out of
            # the head of the queue cuts the PE's startup DMA wait.
            art_sb = consts.tile([P, KT, ER + E], F16)
            nc.sync.dma_start(out=art_sb[:, 0:4, :], in_=artd[:, 0:4, :])

            w_tiles = {}

            def load_w(ot):
                w_sb = wpool.tile([P, KT, P], F16, tag="w", name="w_sb")
                nc.sync.dma_start(out=w_sb[:], in_=wd[ot])
                w_tiles[ot] = w_sb

            # Resident activations: x^T tiled (p=i%128, k=i//128, t), fp16, 8 MiB.
            # W0 right after x k0: the k-interleaved startup loop consumes
            # W0's k-tiles from its 4th matmul on.
            x_sb = consts.tile([P, KT, TPC], F16)
            nc.sync.dma_start(out=x_sb[:, 0, :], in_=xd[:, 0, :])
            load_w(0)
            nc.sync.dma_start(out=art_sb[:, 4:, :], in_=artd[:, 4:, :])
            for k in range(1, 20):
                nc.sync.dma_start(out=x_sb[:, k, :], in_=xd[:, k, :])
            # W1/sel/bt aren't consumed until ~30us/~45us in; keeping them
            # behind the first 20 x k-tiles keeps the startup loop x-fed
            load_w(1)
            sel_sb = consts.tile([E, ER], F16)
            nc.sync.dma_start(out=sel_sb[:], in_=seld[:])
            for k in range(20, 25):
                nc.sync.dma_start(out=x_sb[:, k, :], in_=xd[:, k, :])
            bt_sb = consts.tile([ROWB + ERA, OUT], F16)
            nc.sync.dma_start(out=bt_sb[:], in_=btd[:])
            for k in range(25, KT):
                nc.sync.dma_start(out=x_sb[:, k, :], in_=xd[:, k, :])

            ones_e1 = consts.tile([E, 1], F16)
            nc.vector.memset(ones_e1[:], 1.0)
            ones_1e = consts.tile([1, E], F16)
            nc.vector.memset(ones_1e[:], 1.0)
            # Gated low-rank projection, fp16. Slab 0 lives in rows 0..31
            # (+ones row 32), slab 1 in rows 64..95 (+ones row 96) so the two
            # lora matmuls occupy disjoint PE row groups and run concurrently.
            wp_sb = consts.tile([ROWB + ERA, TPC], F16)
            nc.vector.memset(wp_sb[ER : ER + 1, :], 1.0)
            nc.vector.memset(wp_sb[ROWB + ER : ROWB + ERA, :], 1.0)

            # ---------- phase 1: proj + router matmul PSUM tiles ----------
            # Slab 0's proj output sits at PSUM partitions 0..35 (PE col group
            # 0), slab 1's at partitions 64..99 of its own bank (col group
            # 64): the two matmuls share the art stationary but occupy
            # disjoint PE column groups, so they stream concurrently.
            # Per slab: rows +0..31 = proj^T (er), rows +32..35 router logits.
            pp_tiles = [
                psum_proj.tile(
                    [t * ROWB + ER + E, NSLAB], F32, tag=f"pp{t}", name=f"pp{t}"
                )
                for t in range(NS_)
            ]
            pps = [pp_tiles[t][t * ROWB : t * ROWB + ER + E, :] for t in range(NS_)]

            # ---------- gating: softmax over the 4 expert rows ----------
            # (no max-sub: |logit| < ~8). Partition reductions/broadcasts are
            # tiny PE matmuls (fp16 operands: fp32 ran at 1/4 rate). The chain
            # is a serial PE->DVE->PE ping-pong (sum -> recip -> bcast -> mul
            # -> select), so it is emitted in STAGES with slices of o-tile 1's
            # k-loop between them: every cross-engine hop hides under ~1.7us
            # of base matmuls instead of stalling the in-order PE.
            #
            # All six mini-matmul outputs share ONE manually-sliced PSUM bank
            # at 32-aligned partition offsets (slab 0 at 0..35, slab 1 at
            # 64..99 -> disjoint PE col groups, concurrent). This is safe
            # because each mini-matmul is single-shot (start+stop): a later
            # start's whole-bank has_written clear resets accumulate semantics
            # only, never stored data, and the one region overlap (g32 over
            # s) is ordered by the true dependency chain through recip.
            ga = [dict() for _ in range(NS_)]

            def gat_exp(t):
                g = ga[t]
                g["e"] = small.tile([E, NSLAB], F16, tag="e", name="e_sb")
                nc.scalar.activation(
                    g["e"][:],
                    pps[t][ER : ER + E, :],
                    mybir.ActivationFunctionType.Exp,
                )

            def gat_sum(t, gat_ps):
                g = ga[t]
                g["s"] = gat_ps[t * ROWB : t * ROWB + 1, :]
                nc.tensor.matmul(g["s"], ones_e1[:], g["e"][:])  # sum_e exp

            def gat_recip(t):
                g = ga[t]
                g["r"] = small.tile([1, NSLAB], F16, tag="r", name="r_sb")
                with nc.allow_low_precision(
                    reason="softmax gates are O(1) and scale only the small "
                    "lora term; fp16 is ample"
                ):
                    nc.vector.reciprocal(g["r"][:], g["s"])

            def gat_r4(t, gat_ps):
                g = ga[t]
                g["r4"] = gat_ps[t * ROWB + 32 : t * ROWB + 32 + E, :]
                # bcast to 4 rows; explicit tile_position (auto-derivation
                # rejects output base partition 96)
                nc.tensor.matmul(
                    g["r4"],
                    ones_1e[:],
                    g["r"][:],
                    tile_position=(0, t * ROWB + 32),
                )

            def gat_g4(t):
                g = ga[t]
                g["g4"] = small.tile([E, NSLAB], F16, tag="g4", name="g4_sb")
                nc.vector.tensor_mul(g["g4"][:], g["e"][:], g["r4"])

            def gat_g32(t, gat_ps):
                # (SCALE * gate)[er, t] via 0/1*SCALE selection matmul
                g = ga[t]
                g["g32"] = gat_ps[t * ROWB : t * ROWB + ER, :]
                nc.tensor.matmul(g["g32"], sel_sb[:], g["g4"][:])

            def gat_wp(t):
                g = ga[t]
                tsl = slice(t * NSLAB, (t + 1) * NSLAB)
                rb = t * ROWB  # slab-1 wproj lives at partition offset 64
                # walrus: tensor_tensor may read at most one operand from PSUM
                g32_sb = small.tile([ER, NSLAB], F32, tag="g32s", name="g32_sb")
                nc.vector.tensor_copy(g32_sb[:], g["g32"])
                nc.vector.tensor_mul(
                    wp_sb[rb : rb + ER, tsl], pps[t][0:ER, :], g32_sb[:]
                )

            # ---------- phase 2: base matmul + lora + bias ----------
            def alloc_pots():
                return [
                    psum_base.tile([P, NSLAB], F32, tag=f"po{t}", name=f"po{t}")
                    for t in range(NS_)
                ]

            def kloop_range(ot, pots, k0, k1):
                for k in range(k0, k1):
                    for t in range(NS_):
                        nc.tensor.matmul(
                            pots[t][:],
                            w_tiles[ot][:, k, :],
                            x_sb[:, k, t * NSLAB : (t + 1) * NSLAB],
                            start=(k == 0),
                            stop=False,
                        )

            def base_kloop(ot):
                if ot not in w_tiles:
                    load_w(ot)
                pots = alloc_pots()
                kloop_range(ot, pots, 0, KT)
                return pots

            def base_tail(ot, pots, chunks=1):
                # per slab: finish the accumulation (lora+bias row), copy the
                # PSUM bank out, and DMA that half immediately — keeps the
                # end-of-kernel drain to half an o-tile instead of a full one.
                # The very last o-tile drains in quarter-slab chunks so the
                # final copy+DMA exposure after the last matmul is minimal.
                osl = slice(ot * P, (ot + 1) * P)
                o_sb = opool.tile([P, TPC], F32, tag="o", name="o_sb")
                for t in range(NS_):
                    tsl = slice(t * NSLAB, (t + 1) * NSLAB)
                    rb = t * ROWB
                    # slab 0 in PE rows 0..32, slab 1 in rows 64..96: disjoint
                    # row groups, so the two 512-cycle matmuls overlap
                    nc.tensor.matmul(
                        pots[t][:],
                        bt_sb[rb : rb + ERA, osl],
                        wp_sb[rb : rb + ERA, tsl],
                        start=False,
                        stop=True,
                    )
                    for c in range(chunks):
                        w = NSLAB // chunks
                        csl = slice(t * NSLAB + c * w, t * NSLAB + (c + 1) * w)
                        psl = slice(c * w, (c + 1) * w)
                        # alternate copy engines so slab 1's copies don't
                        # queue behind slab 0's on the DVE
                        if t == 0:
                            nc.vector.tensor_copy(o_sb[:, csl], pots[t][:, psl])
                        else:
                            nc.scalar.copy(o_sb[:, csl], pots[t][:, psl])
                        nc.sync.dma_start(out=od[osl, csl], in_=o_sb[:, csl])
                del w_tiles[ot]

            for rep in range(reps):
                if rep == 0:
                    # k-interleaved startup: the proj/router and o-tile-0
                    # matmuls share each x k-tile, so the PE tracks the x DMA
                    # stream instead of running dry; o-tile 1's k-loop follows
                    # un-interleaved to cover the gating chain's ACT/DVE
                    # latency before the o-tile-0/1 lora tails need wp_sb.
                    pots0 = [
                        psum_base.tile([P, NSLAB], F32, tag=f"po{t}", name=f"po{t}")
                        for t in range(NS_)
                    ]
                    LAG = 6  # o-tile 0's k-index trails proj's so its
                    # first matmul reaches the PE only after W0's DMA lands
                    for k in range(KT + LAG):
                        if k < KT:
                            for t in range(NS_):
                                # slab 0 -> array cols 0..35, slab 1 -> 64..99
                                nc.tensor.matmul(
                                    pps[t][:],
                                    art_sb[:, k, :],
                                    x_sb[:, k, t * NSLAB : (t + 1) * NSLAB],
                                    start=(k == 0),
                                    stop=(k == KT - 1),
                                )
                        if k >= LAG:
                            kb = k - LAG
                            for t in range(NS_):
                                nc.tensor.matmul(
                                    pots0[t][:],
                                    w_tiles[0][:, kb, :],
                                    x_sb[:, kb, t * NSLAB : (t + 1) * NSLAB],
                                    start=(kb == 0),
                                    stop=False,
                                )
                    # prefetch the next W tiles NOW: later in the DMA queue
                    # they'd sit behind the o-tile-0/1 output DMAs, which
                    # can't start until their PSUM copies (~45us in)
                    load_w(2)
                    load_w(3)
                    # the shared mini-matmul bank (see gating comment above)
                    gat_ps = psum_proj.tile(
                        [ROWB + 32 + E, NSLAB], F32, tag="gat", name="gat_ps"
                    )
                    for t in range(NS_):
                        gat_exp(t)
                    # o-tile 1's k-loop in slices between gating stages: each
                    # PE->DVE->PE hop of the softmax chain hides under ~1.7us
                    # of base matmuls instead of stalling the in-order PE
                    pots1 = alloc_pots()
                    kloop_range(1, pots1, 0, 8)
                    for t in range(NS_):
                        gat_sum(t, gat_ps)
                    for t in range(NS_):
                        gat_recip(t)
                    kloop_range(1, pots1, 8, 16)
                    for t in range(NS_):
                        gat_r4(t, gat_ps)
                    for t in range(NS_):
                        gat_g4(t)
                    kloop_range(1, pots1, 16, 24)
                    for t in range(NS_):
                        gat_g32(t, gat_ps)
                    for t in range(NS_):
                        gat_wp(t)
                    kloop_range(1, pots1, 24, KT)
                    base_tail(0, pots0)
                    base_tail(1, pots1)
                    start_ot = 2
                else:
                    start_ot = 0
                for ot in range(start_ot, OT):
                    # keep W two o-tiles ahead of the PE (nearest-first)
                    for o2 in (ot, ot + 1, ot + 2):
                        if o2 < OT and o2 not in w_tiles:
                            load_w(o2)
                    pots = base_kloop(ot)
                    base_tail(ot, pots)

    nc.compile()
    if DEDUPE_LDW:
        _dedupe_ldweights(nc)
    return nc


def get_nc():
    global _NC
    if _NC is None:
        _NC = build_nc()
    return _NC


def _prep_shared(W, b, A, Bm, Wr):
    # W (OUT, IN) -> wd[ot, p, k, o] = W[ot*128+o, k*128+p], fp16, contiguous
    wd = np.ascontiguousarray(
        W.reshape(OT, P, KT, P).transpose(0, 3, 2, 1).astype(np.float16)
    )
    # [A (E,R,IN) flattened; Wr (E,IN)] -> art[p, k, j] = AR[j, k*128+p]
    ar = np.concatenate([A.reshape(ER, IN), Wr], axis=0)  # (36, IN)
    artd = np.ascontiguousarray(
        ar.T.reshape(KT, P, ER + E).transpose(1, 0, 2).astype(np.float16)
    )
    # Bcat rows er = Bm[e,:,r]; row 32 = bias. Duplicated at row offset 64 so
    # the two token slabs' lora matmuls can use disjoint PE row groups.
    bt = np.concatenate([Bm.transpose(0, 2, 1).reshape(ER, OUT), b[None, :]], axis=0)
    btd = np.zeros((ROWB + ERA, OUT), np.float16)
    btd[0:ERA] = bt.astype(np.float16)
    btd[ROWB : ROWB + ERA] = btd[0:ERA]
    sel = np.zeros((E, ER), np.float16)
    for e in range(E):
        sel[e, e * R : (e + 1) * R] = SCALE
    return wd, artd, btd, sel


def _prep_x_shard(xt, c):
    xs = xt[c * TPC : (c + 1) * TPC]  # (TPC, IN)
    return np.ascontiguousarray(
        xs.T.reshape(KT, P, TPC).transpose(1, 0, 2).astype(np.float16)
    )


def make_in_maps(x, W, b, A, Bm, Wr):
    xt = np.asarray(x, np.float32).reshape(TOK, IN)
    wd, artd, btd, sel = _prep_shared(
        np.asarray(W, np.float32),
        np.asarray(b, np.float32),
        np.asarray(A, np.float32),
        np.asarray(Bm, np.float32),
        np.asarray(Wr, np.float32),
    )
    return [
        {
            "xd": _prep_x_shard(xt, c),
            "wd": wd,
            "artd": artd,
            "btd": btd,
            "seld": sel,
        }
        for c in range(N_CORES)
    ]


def gather_out(results):
    # per-core od is (OUT, TPC); tokens are sharded contiguously
    return np.concatenate([r["od"].T for r in results], axis=0).reshape(B, S, OUT)


def kernel(x, W, b, A, Bm, Wr, _trace=False):
    nc = get_nc()
    in_maps = make_in_maps(x, W, b, A, Bm, Wr)
    res = bass_utils.run_bass_kernel_spmd(
        nc, in_maps, core_ids=list(range(N_CORES)), trace=_trace
    )
    out = gather_out(res.results)
    if _trace:
        return out, res
    return out



# revision 59
# speedup vs baseline: 1.3833x; 1.0219x over previous
"""MoE QLoRA linear kernel for Trainium2 (8 NeuronCores, data-parallel over tokens).

Computes, for x:(B,S,IN) f32:
    base  = x @ W.T + b
    gates = softmax(x @ Wr.T)                       # (tok, E)
    proj  = x @ A[e].T                              # (tok, E, R)
    out   = base + sum_e SCALE * gates[...,e] * (proj[...,e,:] @ Bm[e].T)

Key algebraic fold: the gated expert mix is a single rank-(E*R) matmul:
    wproj[t, er] = SCALE * gates[t, e] * proj[t, er]          (er = e*R+r)
    lora[t, o]   = sum_er wproj[t, er] * Bcat[er, o]          (Bcat[er,o] = Bm[e,o,r])
and the bias b is folded in as an extra contraction row (wproj row of ones,
Bcat row = b), so base+lora+bias all accumulate in one PSUM group on the PE.

Per-core kernel (1024 tokens), everything oriented (feature-partition, token-free):
  phase 1: PSUM(36,512) = [A;Wr]^T-stationary matmuls over 32 k-tiles ->
           proj rows 0..31, router logits rows 32..35 (col-tiled: the two
           token slabs stream concurrently); softmax via exp + staged PE
           ones-matmul partition reductions/broadcasts; wproj written fp16.
           The o-tile-0 base k-loop is interleaved into the proj k-loop
           (k-index staggered behind it) so the PE tracks the x DMA stream,
           and o-tile 1's k-loop is sliced between the gating stages.
  phase 2: for each of 32 o-tiles: out(128o, t) = W-tile-stationary matmul
           over 32 k-tiles + one lora matmul (k=33) accumulated into PSUM,
           copy to SBUF, DMA out as (OUT, tok); host transposes back.

All matmul inputs are fp16 (host-cast; PE runs fp16 at full bf16 rate,
fp32 PSUM accumulation). Host pre-tiles all layouts so every DMA is
contiguous and the kernel needs zero on-chip transposes.

Optimizations landed on top of the first working version (600us/core):
 - LDWEIGHTS dedupe (_dedupe_ldweights): walrus runs --enable-ldw-opt=false
   (=true crashes its codegen on our standalone LDWs), so each matmul gets
   its own LDWEIGHTS; the second load of each same-stationary (o-tile, k)
   pair is redundant. Deleting them post-compile is hardware-validated and
   worth ~30us/rep (probe-measured). The signature includes tile_position/
   tile_size, and any self-loading (fp32) matmul resets it — without that,
   two deletions with an interleaved gating matmul corrupted o-tile 1.
 - fp16 gating matmuls (fp32 ran at 1/4 PE rate): ~4us.
 - Row-tiled lora tails: slab 0 in PE rows 0..32, slab 1 in rows 64..96
   (bt/wp duplicated at partition offset 64), so the two 512-cycle lora
   matmuls stream concurrently: ~7us.
 - Col-tiled phase-1 proj: slab 0 -> PSUM partitions 0..35 (PE col group 0),
   slab 1 -> partitions 64..99 of its own bank (col group 64), sharing the
   art stationary: ~7us.
 - DMA issue order tracks PE consumption (art k0-3, x k0, W0, ... W two
   o-tiles ahead of the PE), and the startup k-loop interleaves proj with
   o-tile 0 so the PE follows the x DMA stream: ~6us of startup idle.
 - Staged gating: the softmax chain is a serial PE->DVE->PE ping-pong, so
   its stages are emitted with slices of o-tile 1's k-loop between them
   (every cross-engine hop hides under ~1.7us of base matmuls), and all six
   mini-matmul outputs share one manually-sliced PSUM bank at 32-aligned
   partition offsets (single-shot matmuls: a later start's whole-bank
   has_written clear resets accumulate semantics, never stored data): ~4us.

Measured device behavior (pure-matmul probe, ldw_probe.py): the chip
oscillates between ~2.4GHz and ~2.0GHz PE power states with ~0.1-1s dwell;
per-rep slope is ~446us fast / ~528us slow with dedupe (the fp16 streaming
roofline for 2112 N=512 matmuls is 450us at 2.4GHz, i.e. LDWEIGHTS is fully
hidden after dedupe). test.py therefore reports the median per-rep slope
over many short alternating loops, scaled by the cost-model full/base ratio.
"""

import numpy as np

import concourse.bass as bass
import concourse.tile as tile
from concourse import bacc, mybir
from concourse import bass_utils

# Problem shape (hardcoded; kernel.py must be self-contained)
B, S, IN, OUT, E, R = 4, 2048, 4096, 4096, 4, 8
SCALE = 16.0 / 8.0
N_CORES = 8
TOK = B * S                  # 8192 tokens
TPC = TOK // N_CORES         # 1024 tokens per core
P = 128                      # partitions
KT = IN // P                 # 32 k-tiles (contraction)
OT = OUT // P                # 32 output tiles
NSLAB = 512                  # moving-operand free size (PSUM bank = 512 f32)
NS = TPC // NSLAB            # 2 token slabs per core
ER = E * R                   # 32 low-rank rows
ERA = ER + 1                 # +1 ones row (bias fold)
ROWB = 64                    # partition offset of slab-1's lora row-tile

F16 = mybir.dt.float16
F32 = mybir.dt.float32

_NC = None

# Post-compile pass: delete redundant consecutive InstLdweights from the PE
# stream. Walrus (--enable-ldw-opt=false) emits one LDWEIGHTS per matmul;
# when consecutive matmuls share the same stationary tile (the two token
# slabs of each (o-tile, k)), the second load is identical, wait/update-free,
# and costs ~53ns of serialized PE time. Deleting it lets the following
# non-self-loading matmul reuse the already-loaded weights.
DEDUPE_LDW = True


def _dedupe_ldweights(nc):
    removed = 0
    for blk in nc.m.functions[0].blocks:
        keep = []
        prev_sig = None
        for inst in blk.instructions:
            if isinstance(inst, mybir.InstLdweights):
                sig = (
                    str(inst.ins[0]),
                    str(inst.tile_position),
                    str(inst.tile_size),
                    str(inst.perf_mode),
                    str(inst.is_transpose),
                )
                if sig == prev_sig and not inst.has_wait() and not inst.has_update():
                    removed += 1
                    continue
                prev_sig = sig
            elif isinstance(inst, mybir.InstMatmult) and inst.ldweights is not False:
                # self-loading matmul (fp32 gating) reloads the PE array's
                # stationary weights — following matmuls need a fresh LDW
                prev_sig = None
            keep.append(inst)
        blk.instructions = keep
    return removed


def build_nc(reps=1, ns=NS):
    NS_ = ns
    nc = bacc.Bacc("TRN2", target_bir_lowering=False, debug=False)

    xd = nc.dram_tensor("xd", [P, KT, TPC], F16, kind="ExternalInput")
    wd = nc.dram_tensor("wd", [OT, P, KT, P], F16, kind="ExternalInput")
    artd = nc.dram_tensor("artd", [P, KT, ER + E], F16, kind="ExternalInput")
    # btd rows 0..32 = [Bcat; bias]; rows 64..96 = same again, so the two
    # slabs' lora matmuls can run concurrently as PE row-tiles (0-63 / 64-127)
    btd = nc.dram_tensor("btd", [ROWB + ERA, OUT], F16, kind="ExternalInput")
    seld = nc.dram_tensor("seld", [E, ER], F16, kind="ExternalInput")
    od = nc.dram_tensor("od", [OUT, TPC], F32, kind="ExternalOutput")

    with tile.TileContext(nc) as tc:
        with (
            tc.tile_pool(name="consts", bufs=1) as consts,
            tc.tile_pool(name="wpool", bufs=4) as wpool,
            tc.tile_pool(name="opool", bufs=3) as opool,
            tc.tile_pool(name="small", bufs=2) as small,
            tc.tile_pool(name="psum_proj", bufs=1, space="PSUM") as psum_proj,
            tc.tile_pool(name="psum_base", bufs=2, space="PSUM") as psum_base,
        ):
            # DMA issue order tracks the PE consumption schedule: phase 1
            # consumes art k-tiles + x k-tiles in order from t=0; W tile 0
            # isn't needed until the o-tile-0 k-loop (~15us in), W tile 1 and
            # the bias/sel tiles later still. Keeping the big W loads out of
            # the head of the queue cuts the PE's startup DMA wait.
            art_sb = consts.tile([P, KT, ER + E], F16)
            nc.sync.dma_start(out=art_sb[:, 0:4, :], in_=artd[:, 0:4, :])

            w_tiles = {}

            def load_w(ot):
                w_sb = wpool.tile([P, KT, P], F16, tag="w", name="w_sb")
                nc.sync.dma_start(out=w_sb[:], in_=wd[ot])
                w_tiles[ot] = w_sb

            # Resident activations: x^T tiled (p=i%128, k=i//128, t), fp16, 8 MiB.
            # W0 right after x k0: the k-interleaved startup loop consumes
            # W0's k-tiles from its 4th matmul on.
            x_sb = consts.tile([P, KT, TPC], F16)
            nc.sync.dma_start(out=x_sb[:, 0, :], in_=xd[:, 0, :])
            load_w(0)
            nc.sync.dma_start(out=art_sb[:, 4:, :], in_=artd[:, 4:, :])
            for k in range(1, 20):
                nc.sync.dma_start(out=x_sb[:, k, :], in_=xd[:, k, :])
            # W1/sel/bt aren't consumed until ~30us/~45us in; keeping them
            # behind the first 20 x k-tiles keeps the startup loop x-fed
            load_w(1)
            sel_sb = consts.tile([E, ER], F16)
            nc.sync.dma_start(out=sel_sb[:], in_=seld[:])
            for k in range(20, 25):
                nc.sync.dma_start(out=x_sb[:, k, :], in_=xd[:, k, :])
            bt_sb = consts.tile([ROWB + ERA, OUT], F16)
            nc.sync.dma_start(out=bt_sb[:], in_=btd[:])
            for k in range(25, KT):
                nc.sync.dma_start(out=x_sb[:, k, :], in_=xd[:, k, :])

            ones_e1 = consts.tile([E, 1], F16)
            nc.vector.memset(ones_e1[:], 1.0)
            ones_1e = consts.tile([1, E], F16)
            nc.vector.memset(ones_1e[:], 1.0)
            # Gated low-rank projection, fp16. Slab 0 lives in rows 0..31
            # (+ones row 32), slab 1 in rows 64..95 (+ones row 96) so the two
            # lora matmuls occupy disjoint PE row groups and run concurrently.
            wp_sb = consts.tile([ROWB + ERA, TPC], F16)
            nc.vector.memset(wp_sb[ER : ER + 1, :], 1.0)
            nc.vector.memset(wp_sb[ROWB + ER : ROWB + ERA, :], 1.0)

            # ---------- phase 1: proj + router matmul PSUM tiles ----------
            # Slab 0's proj output sits at PSUM partitions 0..35 (PE col group
            # 0), slab 1's at partitions 64..99 of its own bank (col group
            # 64): the two matmuls share the art stationary but occupy
            # disjoint PE column groups, so they stream concurrently.
            # Per slab: rows +0..31 = proj^T (er), rows +32..35 router logits.
            pp_tiles = [
                psum_proj.tile(
                    [t * ROWB + ER + E, NSLAB], F32, tag=f"pp{t}", name=f"pp{t}"
                )
                for t in range(NS_)
            ]
            pps = [pp_tiles[t][t * ROWB : t * ROWB + ER + E, :] for t in range(NS_)]

            # ---------- gating: softmax over the 4 expert rows ----------
            # (no max-sub: |logit| < ~8). Partition reductions/broadcasts are
            # tiny PE matmuls (fp16 operands: fp32 ran at 1/4 rate). The chain
            # is a serial PE->DVE->PE ping-pong (sum -> recip -> bcast -> mul
            # -> select), so it is emitted in STAGES with slices of o-tile 1's
            # k-loop between them: every cross-engine hop hides under ~1.7us
            # of base matmuls instead of stalling the in-order PE.
            #
            # All six mini-matmul outputs share ONE manually-sliced PSUM bank
            # at 32-aligned partition offsets (slab 0 at 0..35, slab 1 at
            # 64..99 -> disjoint PE col groups, concurrent). This is safe
            # because each mini-matmul is single-shot (start+stop): a later
            # start's whole-bank has_written clear resets accumulate semantics
            # only, never stored data, and the one region overlap (g32 over
            # s) is ordered by the true dependency chain through recip.
            ga = [dict() for _ in range(NS_)]

            def gat_exp(t):
                g = ga[t]
                g["e"] = small.tile([E, NSLAB], F16, tag="e", name="e_sb")
                nc.scalar.activation(
                    g["e"][:],
                    pps[t][ER : ER + E, :],
                    mybir.ActivationFunctionType.Exp,
                )

            def gat_sum(t, gat_ps):
                g = ga[t]
                g["s"] = gat_ps[t * ROWB : t * ROWB + 1, :]
                nc.tensor.matmul(g["s"], ones_e1[:], g["e"][:])  # sum_e exp

            def gat_recip(t):
                g = ga[t]
                g["r"] = small.tile([1, NSLAB], F16, tag="r", name="r_sb")
                with nc.allow_low_precision(
                    reason="softmax gates are O(1) and scale only the small "
                    "lora term; fp16 is ample"
                ):
                    nc.vector.reciprocal(g["r"][:], g["s"])

            def gat_r4(t, gat_ps):
                g = ga[t]
                g["r4"] = gat_ps[t * ROWB + 32 : t * ROWB + 32 + E, :]
                # bcast to 4 rows; explicit tile_position (auto-derivation
                # rejects output base partition 96)
                nc.tensor.matmul(
                    g["r4"],
                    ones_1e[:],
                    g["r"][:],
                    tile_position=(0, t * ROWB + 32),
                )

            def gat_g4(t):
                g = ga[t]
                g["g4"] = small.tile([E, NSLAB], F16, tag="g4", name="g4_sb")
                nc.vector.tensor_mul(g["g4"][:], g["e"][:], g["r4"])

            def gat_g32(t, gat_ps):
                # (SCALE * gate)[er, t] via 0/1*SCALE selection matmul
                g = ga[t]
                g["g32"] = gat_ps[t * ROWB : t * ROWB + ER, :]
                nc.tensor.matmul(g["g32"], sel_sb[:], g["g4"][:])

            def gat_wp(t):
                g = ga[t]
                tsl = slice(t * NSLAB, (t + 1) * NSLAB)
                rb = t * ROWB  # slab-1 wproj lives at partition offset 64
                # walrus: tensor_tensor may read at most one operand from PSUM
                g32_sb = small.tile([ER, NSLAB], F32, tag="g32s", name="g32_sb")
                nc.vector.tensor_copy(g32_sb[:], g["g32"])
                nc.vector.tensor_mul(
                    wp_sb[rb : rb + ER, tsl], pps[t][0:ER, :], g32_sb[:]
                )

            # ---------- phase 2: base matmul + lora + bias ----------
            def alloc_pots():
                return [
                    psum_base.tile([P, NSLAB], F32, tag=f"po{t}", name=f"po{t}")
                    for t in range(NS_)
                ]

            def kloop_range(ot, pots, k0, k1):
                for k in range(k0, k1):
                    for t in range(NS_):
                        nc.tensor.matmul(
                            pots[t][:],
                            w_tiles[ot][:, k, :],
                            x_sb[:, k, t * NSLAB : (t + 1) * NSLAB],
                            start=(k == 0),
                            stop=False,
                        )

            def base_kloop(ot):
                if ot not in w_tiles:
                    load_w(ot)
                pots = alloc_pots()
                kloop_range(ot, pots, 0, KT)
                return pots

            def base_tail(ot, pots, chunks=1):
                # per slab: finish the accumulation (lora+bias row), copy the
                # PSUM bank out, and DMA that half immediately — keeps the
                # end-of-kernel drain to half an o-tile instead of a full one.
                # The very last o-tile drains in quarter-slab chunks so the
                # final copy+DMA exposure after the last matmul is minimal.
                osl = slice(ot * P, (ot + 1) * P)
                o_sb = opool.tile([P, TPC], F32, tag="o", name="o_sb")
                for t in range(NS_):
                    tsl = slice(t * NSLAB, (t + 1) * NSLAB)
                    rb = t * ROWB
                    # slab 0 in PE rows 0..32, slab 1 in rows 64..96: disjoint
                    # row groups, so the two 512-cycle matmuls overlap
                    nc.tensor.matmul(
                        pots[t][:],
                        bt_sb[rb : rb + ERA, osl],
                        wp_sb[rb : rb + ERA, tsl],
                        start=False,
                        stop=True,
                    )
                    for c in range(chunks):
                        w = NSLAB // chunks
                        csl = slice(t * NSLAB + c * w, t * NSLAB + (c + 1) * w)
                        psl = slice(c * w, (c + 1) * w)
                        # alternate copy engines so slab 1's copies don't
                        # queue behind slab 0's on the DVE
                        if t == 0:
                            nc.vector.tensor_copy(o_sb[:, csl], pots[t][:, psl])
                        else:
                            nc.scalar.copy(o_sb[:, csl], pots[t][:, psl])
                        nc.sync.dma_start(out=od[osl, csl], in_=o_sb[:, csl])
                del w_tiles[ot]

            for rep in range(reps):
                if rep == 0:
                    # k-interleaved startup: the proj/router and o-tile-0
                    # matmuls share each x k-tile, so the PE tracks the x DMA
                    # stream instead of running dry; o-tile 1's k-loop follows
                    # un-interleaved to cover the gating chain's ACT/DVE
                    # latency before the o-tile-0/1 lora tails need wp_sb.
                    pots0 = [
                        psum_base.tile([P, NSLAB], F32, tag=f"po{t}", name=f"po{t}")
                        for t in range(NS_)
                    ]
                    LAG = 6  # o-tile 0's k-index trails proj's so its
                    # first matmul reaches the PE only after W0's DMA lands
                    for k in range(KT + LAG):
                        if k < KT:
                            for t in range(NS_):
                                # slab 0 -> array cols 0..35, slab 1 -> 64..99
                                nc.tensor.matmul(
                                    pps[t][:],
                                    art_sb[:, k, :],
                                    x_sb[:, k, t * NSLAB : (t + 1) * NSLAB],
                                    start=(k == 0),
                                    stop=(k == KT - 1),
                                )
                        if k >= LAG:
                            kb = k - LAG
                            for t in range(NS_):
                                nc.tensor.matmul(
                                    pots0[t][:],
                                    w_tiles[0][:, kb, :],
                                    x_sb[:, kb, t * NSLAB : (t + 1) * NSLAB],
                                    start=(kb == 0),
                                    stop=False,
                                )
                    # prefetch the next W tiles NOW: later in the DMA queue
                    # they'd sit behind the o-tile-0/1 output DMAs, which
                    # can't start until their PSUM copies (~45us in)
                    load_w(2)
                    load_w(3)
                    # the shared mini-matmul bank (see gating comment above)
                    gat_ps = psum_proj.tile(
                        [ROWB + 32 + E, NSLAB], F32, tag="gat", name="gat_ps"
                    )
                    for t in range(NS_):
                        gat_exp(t)
                    # o-tile 1's k-loop in slices between gating stages: each
                    # PE->DVE->PE hop of the softmax chain hides under ~1.7us
                    # of base matmuls instead of stalling the in-order PE
                    pots1 = alloc_pots()
                    kloop_range(1, pots1, 0, 8)
                    for t in range(NS_):
                        gat_sum(t, gat_ps)
                    for t in range(NS_):
                        gat_recip(t)
                    kloop_range(1, pots1, 8, 16)
                    for t in range(NS_):
                        gat_r4(t, gat_ps)
                    for t in range(NS_):
                        gat_g4(t)
                    kloop_range(1, pots1, 16, 24)
                    for t in range(NS_):
                        gat_g32(t, gat_ps)
                    for t in range(NS_):
                        gat_wp(t)
                    kloop_range(1, pots1, 24, KT)
                    base_tail(0, pots0)
                    base_tail(1, pots1)
                    start_ot = 2
                else:
                    start_ot = 0
                for ot in range(start_ot, OT):
                    # keep W two o-tiles ahead of the PE (nearest-first)
                    for o2 in (ot, ot + 1, ot + 2):
                        if o2 < OT and o2 not in w_tiles:
                            load_w(o2)
                    pots = base_kloop(ot)
                    base_tail(ot, pots)

    nc.compile()
    if DEDUPE_LDW:
        _dedupe_ldweights(nc)
    return nc


def get_nc():
    global _NC
    if _NC is None:
        _NC = build_nc()
    return _NC


def _prep_shared(W, b, A, Bm, Wr):
    # W (OUT, IN) -> wd[ot, p, k, o] = W[ot*128+o, k*128+p], fp16, contiguous
    wd = np.ascontiguousarray(
        W.reshape(OT, P, KT, P).transpose(0, 3, 2, 1).astype(np.float16)
    )
    # [A (E,R,IN) flattened; Wr (E,IN)] -> art[p, k, j] = AR[j, k*128+p]
    ar = np.concatenate([A.reshape(ER, IN), Wr], axis=0)  # (36, IN)
    artd = np.ascontiguousarray(
        ar.T.reshape(KT, P, ER + E).transpose(1, 0, 2).astype(np.float16)
    )
    # Bcat rows er = Bm[e,:,r]; row 32 = bias. Duplicated at row offset 64 so
    # the two token slabs' lora matmuls can use disjoint PE row groups.
    bt = np.concatenate([Bm.transpose(0, 2, 1).reshape(ER, OUT), b[None, :]], axis=0)
    btd = np.zeros((ROWB + ERA, OUT), np.float16)
    btd[0:ERA] = bt.astype(np.float16)
    btd[ROWB : ROWB + ERA] = btd[0:ERA]
    sel = np.zeros((E, ER), np.float16)
    for e in range(E):
        sel[e, e * R : (e + 1) * R] = SCALE
    return wd, artd, btd, sel


def _prep_x_shard(xt, c):
    xs = xt[c * TPC : (c + 1) * TPC]  # (TPC, IN)
    return np.ascontiguousarray(
        xs.T.reshape(KT, P, TPC).transpose(1, 0, 2).astype(np.float16)
    )


def make_in_maps(x, W, b, A, Bm, Wr):
    xt = np.asarray(x, np.float32).reshape(TOK, IN)
    wd, artd, btd, sel = _prep_shared(
        np.asarray(W, np.float32),
        np.asarray(b, np.float32),
        np.asarray(A, np.float32),
        np.asarray(Bm, np.float32),
        np.asarray(Wr, np.float32),
    )
    return [
        {
            "xd": _prep_x_shard(xt, c),
            "wd": wd,
            "artd": artd,
            "btd": btd,
            "seld": sel,
        }
        for c in range(N_CORES)
    ]


def gather_out(results):
    # per-core od is (OUT, TPC); tokens are sharded contiguously
    return np.concatenate([r["od"].T for r in results], axis=0).reshape(B, S, OUT)


def kernel(x, W, b, A, Bm, Wr, _trace=False):
    nc = get_nc()
    in_maps = make_in_maps(x, W, b, A, Bm, Wr)
    res = bass_utils.run_bass_kernel_spmd(
        nc, in_maps, core_ids=list(range(N_CORES)), trace=_trace
    )
    out = gather_out(res.results)
    if _trace:
        return out, res
    return out

